# revision 43
# baseline (speedup 1.0000x reference)
"""Trainium2 Bass kernel for nn_MembraneLayer: h = x @ w followed by a
double first-order recurrence over time, producing (syn_rec, mem_rec).

Sharding: data-parallel over batch. 8 cores x 64 batches each.

Per-core layout (hardcoded), columns = b*100 + t per quartet of 16
batches (4 batches per PSUM bank):
  xt16  [700, 6400]   fp16  x transposed to [C, b*T+t]; t=99 cols zeroed on
                            the host (they feed the PSUM t=0 zeroing matmul)
  w16   [700, 512]    fp16
  acoef [4, 128, 400] f32   alpha per (d_tile, partition), 0 at t=0 cols
  bcoef [4, 128, 400] f32   beta likewise (scans run per 400-col bank)
  syn   [512, 6400]   fp16  out: syn_rec in [D, b*T+t] layout
  mem   [512, 6400]   fp16  out: v = mem/(1-beta), UNSHIFTED; host applies
                            the (1-beta) scale and the one-step shift

Design notes (measured on HW):
 - single fp16 matmul pass (PE streams ~0.42 ns/col; 6 k-tiles x 4 banks
   per (quartet, d_tile), shifted write to slot t+1, t=0 memset)
 - the DVE serial scan costs ~2.2 ns/col and supports NO 2x perf modes
   (InstTensorScalarPtr reports none); a Blelloch block-scan pyramid of
   full-rate STT ops is NOT cheaper because STT also has no 2x mode and
   per-op overhead (~130 ns) eats the difference: plain scans with
   minimal op count win. GPSIMD cannot run TensorScalarPtr at all.
 - fp16 outputs halve store traffic; scan state stays fp32 internally
   (out is downcast on write), so precision loss is only on stored values
 - scan coefficients come from DRAM (DMA has slack; DVE is the
   bottleneck, so no on-device coef generation)
"""

import os
from contextlib import ExitStack

import numpy as np

import concourse.bass as bass
import concourse.tile as tile
from concourse import bacc, mybir
from concourse import bass_utils

B, T, C, D = 512, 100, 700, 512
NCORES = 8
BC = B // NCORES  # 64 batches per core
NQ = 4  # quartets: 16 batches = 1600 columns each
QCOLS = 1600
KT = [(k * 128, min(128, C - k * 128)) for k in range(6)]
F32 = mybir.dt.float32
FP16 = mybir.dt.float16
MULT = mybir.AluOpType.mult
ADD = mybir.AluOpType.add
COPY = mybir.ActivationFunctionType.Copy

MODE = "fp16-plainscan"
LAST_RESULT = None
_cache = {}


def _build(sim_safe=False):
    """sim_safe=True splits each matmul per batch (rank-2 out views) so
    CoreSim's 2-D result assert passes; numerics identical."""
    key = ("nc", sim_safe)
    if key in _cache:
        return _cache[key]
    nc = bacc.Bacc("TRN2", target_bir_lowering=False, debug=False)

    xt_d = nc.dram_tensor("xt16", [C, BC * T], FP16, kind="ExternalInput").ap()
    w_d = nc.dram_tensor("w16", [C, D], FP16, kind="ExternalInput").ap()
    ac_d = nc.dram_tensor("acoef", [4, 128, 400], F32, kind="ExternalInput").ap()
    bc_d = nc.dram_tensor("bcoef", [4, 128, 400], F32, kind="ExternalInput").ap()
    syn_d = nc.dram_tensor("syn", [D, BC * T], FP16, kind="ExternalOutput").ap()
    mem_d = nc.dram_tensor("mem", [D, BC * T], FP16, kind="ExternalOutput").ap()

    with tile.TileContext(nc) as tc:
        with ExitStack() as ctx:
            cpool = ctx.enter_context(tc.tile_pool(name="consts", bufs=1))
            # warm-up scratch zeroed on gpsimd (its queue is alive ~1us before
            # the DVE's first op can land), issued ahead of the w DMAs
            warm_sb = cpool.tile([128, 512], FP16, name="warm", tag="warm")
            nc.gpsimd.memset(warm_sb[:], 0.0)
            # weights + coefs on the gpsimd (SWDGE) queue so the Sync queue
            # leads with the first x tiles (A/B-tested vs the scalar queue:
            # scalar was ~5us slower end-to-end)
            w_tiles = []
            for k, (r0_, rk) in enumerate(KT):
                wt = cpool.tile([128, D], FP16, name=f"w{k}", tag=f"w{k}")
                nc.gpsimd.dma_start(wt[:rk, :], w_d[r0_ : r0_ + rk, :])
                w_tiles.append(wt)
            # coef tiles are small (one bank-width each; scans reuse them per
            # bank) so the startup DMA stays light
            ac_t, bc_t = [], []
            for di in range(4):
                a = cpool.tile([128, 400], F32, name=f"ac{di}", tag=f"ac{di}")
                nc.gpsimd.dma_start(a[:], ac_d[di])
                ac_t.append(a)
                b_ = cpool.tile([128, 400], F32, name=f"bc{di}", tag=f"bc{di}")
                nc.gpsimd.dma_start(b_[:], bc_d[di])
                bc_t.append(b_)

            xp = ctx.enter_context(tc.tile_pool(name="xp", bufs=2))
            pp = ctx.enter_context(tc.tile_pool(name="pp", bufs=2, space="PSUM"))
            sp = ctx.enter_context(tc.tile_pool(name="sp", bufs=2))
            vp = ctx.enter_context(tc.tile_pool(name="vp", bufs=2))
            hp = ctx.enter_context(tc.tile_pool(name="hp", bufs=2))

            # ACT warmup: a dummy activation absorbs the ~1.3us ACT table
            # load at boot, so the first unit's h16 staging copy isn't
            # delayed by it
            actw = cpool.tile([128, 1], F32, name="actw", tag="actw")
            nc.scalar.activation(actw[:], warm_sb[:, 0:1], COPY)
            # PE warmup: dummy matmuls run during the initial DMA wait so HAM
            # un-throttles before the first real MM
            warm_ps = pp.tile([128, 2048], F32, tag="ps", name="warm_ps")
            for _ in range(16):
                nc.tensor.matmul(
                    warm_ps[:, 0:512], warm_sb[:, 0:128], warm_sb[:], start=True, stop=True
                )

            for q in range(NQ):
                qc0 = q * QCOLS
                xts = []
                for k, (r0_, rk) in enumerate(KT):
                    t_ = xp.tile([128, QCOLS], FP16, tag=f"x{k}", name=f"x{k}_{q}")
                    nc.sync.dma_start(t_[:rk, :], xt_d[r0_ : r0_ + rk, qc0 : qc0 + QCOLS])
                    xts.append(t_)

                for di in range(4):
                    dsl = slice(di * 128, (di + 1) * 128)

                    # h matmul: 4 batches per PSUM bank, shifted write to t+1.
                    # The t=0 slots are zeroed by an extra k=0 matmul over the
                    # host-zeroed t=99 x columns (avoids a DVE memset + stall).
                    ps = pp.tile([128, 2048], F32, tag="ps", name=f"ps_{q}_{di}")
                    for k, (r0_, rk) in enumerate(KT):
                        lhsT = w_tiles[k][:rk, dsl]
                        for g in range(4):
                            if sim_safe:
                                for b_ in range(4):
                                    c0 = g * 400 + b_ * 100
                                    nc.tensor.matmul(
                                        ps[:, g * 512 + b_ * 100 + 1 : g * 512 + b_ * 100 + 100],
                                        lhsT,
                                        xts[k][:rk, c0 : c0 + 99],
                                        start=(k == 0 and b_ == 0),
                                        stop=(k == 5 and b_ == 3),
                                    )
                                    if k == 0:
                                        nc.tensor.matmul(
                                            ps[:, g * 512 + b_ * 100 : g * 512 + b_ * 100 + 1],
                                            lhsT,
                                            xts[k][:rk, c0 + 99 : c0 + 100],
                                            start=False, stop=False,
                                        )
                                continue
                            rhs3 = xts[k][:rk, g * 400 : (g + 1) * 400].rearrange(
                                "p (b t) -> p b t", t=100
                            )[:, :, 0:99]
                            out3 = ps[:, g * 512 : g * 512 + 400].rearrange(
                                "p (b t) -> p b t", t=100
                            )[:, :, 1:100]
                            nc.tensor.matmul(
                                out3, lhsT, rhs3, start=(k == 0), stop=(k == 5)
                            )
                            if k == 0:
                                z3 = ps[:, g * 512 : g * 512 + 400].rearrange(
                                    "p (b t) -> p b t", t=100
                                )[:, :, 0:1]
                                zr = xts[k][:rk, g * 400 : (g + 1) * 400].rearrange(
                                    "p (b t) -> p b t", t=100
                                )[:, :, 99:100]
                                nc.tensor.matmul(z3, lhsT, zr, start=False, stop=False)

                    # ACT stages h PSUM->SBUF (fp16): DVE scans with a PSUM
                    # input cost ~180ns more each than SBUF-input scans, and
                    # the ACT engine is otherwise idle (copy overlaps the
                    # previous unit's scans)
                    h16 = hp.tile([128, QCOLS], FP16, tag="h16", name=f"h16_{q}_{di}")
                    nc.scalar.activation(
                        h16.rearrange("p (g c) -> p g c", c=400),
                        ps.rearrange("p (g x) -> p g x", x=512)[:, :, 0:400],
                        COPY,
                    )
                    syn16 = sp.tile([128, QCOLS], FP16, tag="syn", name=f"sy_{q}_{di}")
                    v16 = vp.tile([128, QCOLS], FP16, tag="v", name=f"v_{q}_{di}")
                    for g in range(4):
                        nc.vector.tensor_tensor_scan(
                            syn16[:, g * 400 : (g + 1) * 400],
                            ac_t[di][:],
                            h16[:, g * 400 : (g + 1) * 400],
                            0.0,
                            MULT,
                            ADD,
                        )
                    nc.scalar.dma_start(syn_d[dsl, qc0 : qc0 + QCOLS], syn16[:])
                    last_unit = q == NQ - 1 and di == 3
                    for g in range(4):
                        nc.vector.tensor_tensor_scan(
                            v16[:, g * 400 : (g + 1) * 400],
                            bc_t[di][:],
                            syn16[:, g * 400 : (g + 1) * 400],
                            0.0,
                            MULT,
                            ADD,
                        )
                        if last_unit:
                            # quarter-stores so the post-final-scan drain is
                            # one 102 KB transfer instead of 409 KB
                            nc.scalar.dma_start(
                                mem_d[dsl, qc0 + g * 400 : qc0 + (g + 1) * 400],
                                v16[:, g * 400 : (g + 1) * 400],
                            )
                    if not last_unit:
                        nc.scalar.dma_start(mem_d[dsl, qc0 : qc0 + QCOLS], v16[:])

    nc.compile()
    _cache[key] = nc
    return nc


def kernel(inputs, w, alpha, beta):
    global LAST_RESULT
    inputs = np.asarray(inputs, dtype=np.float32)
    w = np.asarray(w, dtype=np.float32)
    alpha = np.asarray(alpha, dtype=np.float32).reshape(-1)
    beta = np.asarray(beta, dtype=np.float32).reshape(-1)

    nc = _build()

    acoef = np.broadcast_to(
        alpha.reshape(4, 128, 1), (4, 128, 400)
    ).astype(np.float32).copy()
    acoef[:, :, 0::100] = 0.0
    bcoef = np.broadcast_to(
        beta.reshape(4, 128, 1), (4, 128, 400)
    ).astype(np.float32).copy()
    bcoef[:, :, 0::100] = 0.0
    w16 = w.astype(np.float16)
    omb = (1.0 - beta).reshape(1, 1, D)

    in_maps = []
    for c in range(NCORES):
        xc = inputs[c * BC : (c + 1) * BC].copy()  # [64, 100, 700]
        xc[:, T - 1, :] = 0.0  # t=99 cols feed the PSUM t=0 zeroing matmul
        xt16 = xc.reshape(BC * T, C).T.astype(np.float16)  # [700, 6400]
        in_maps.append({"xt16": xt16, "w16": w16, "acoef": acoef, "bcoef": bcoef})

    run_kwargs = {}
    if os.environ.get("MEMBRANE_TRACE_DIR"):
        run_kwargs["tmpdir"] = os.environ["MEMBRANE_TRACE_DIR"]
    res = bass_utils.run_bass_kernel_spmd(
        nc, in_maps, core_ids=list(range(NCORES)), **run_kwargs
    )
    LAST_RESULT = res

    syn_full = np.empty((B, T, D), dtype=np.float32)
    mem_full = np.empty((B, T, D), dtype=np.float32)
    for c in range(NCORES):
        r = res.results[c]
        cs = slice(c * BC, (c + 1) * BC)
        syn_full[cs] = (
            r["syn"].astype(np.float32).reshape(D, BC, T).transpose(1, 2, 0)
        )
        vt = r["mem"].astype(np.float32).reshape(D, BC, T).transpose(1, 2, 0)
        mem_full[cs, 1:, :] = vt[:, : T - 1, :] * omb
    syn_full[:, 0, :] = 0.0
    mem_full[:, 0, :] = 0.0
    return (syn_full, mem_full)


# revision 44
# speedup vs baseline: 1.1942x; 1.1942x over previous
"""Trainium2 Bass kernel for nn_MembraneLayer: h = x @ w followed by a
double first-order recurrence over time, producing (syn_rec, mem_rec).

Sharding: data-parallel over batch. 8 cores x 64 batches each.

Per-core layout (hardcoded), columns = b*100 + t per quartet of 16
batches (4 batches per PSUM bank):
  xt16  [700, 6400]   fp16  x transposed to [C, b*T+t]; t=99 cols zeroed on
                            the host (they feed the PSUM t=0 zeroing matmul)
  w16   [700, 512]    fp16
  acoef [4, 128, 400] f32   alpha per (d_tile, partition), 0 at t=0 cols
  bcoef [4, 128, 400] f32   beta likewise (scans run per 400-col bank)
  syn   [512, 6400]   fp16  out: syn_rec in [D, b*T+t] layout
  mem   [512, 6400]   fp16  out: v = mem/(1-beta), UNSHIFTED; host applies
                            the (1-beta) scale and the one-step shift

Design notes (measured on HW):
 - single fp16 matmul pass (PE streams ~0.42 ns/col; 6 k-tiles x 4 banks
   per (quartet, d_tile), shifted write to slot t+1, t=0 memset)
 - the DVE serial scan costs ~2.2 ns/col and supports NO 2x perf modes
   (InstTensorScalarPtr reports none); a Blelloch block-scan pyramid of
   full-rate STT ops is NOT cheaper because STT also has no 2x mode and
   per-op overhead (~130 ns) eats the difference: plain scans with
   minimal op count win. GPSIMD cannot run TensorScalarPtr at all.
 - fp16 outputs halve store traffic; scan state stays fp32 internally
   (out is downcast on write), so precision loss is only on stored values
 - scan coefficients come from DRAM (DMA has slack; DVE is the
   bottleneck, so no on-device coef generation)
"""

import os
from contextlib import ExitStack

import numpy as np

import concourse.bass as bass
import concourse.tile as tile
from concourse import bacc, mybir
from concourse import bass_utils

B, T, C, D = 512, 100, 700, 512
NCORES = 8
BC = B // NCORES  # 64 batches per core
NQ = 4  # quartets: 16 batches = 1600 columns each
QCOLS = 1600
KT = [(k * 128, min(128, C - k * 128)) for k in range(6)]
F32 = mybir.dt.float32
FP16 = mybir.dt.float16
MULT = mybir.AluOpType.mult
ADD = mybir.AluOpType.add
COPY = mybir.ActivationFunctionType.Copy

MODE = "fp16-plainscan"
LAST_RESULT = None
_cache = {}


def _build(sim_safe=False):
    """sim_safe=True splits each matmul per batch (rank-2 out views) so
    CoreSim's 2-D result assert passes; numerics identical."""
    key = ("nc", sim_safe)
    if key in _cache:
        return _cache[key]
    nc = bacc.Bacc("TRN2", target_bir_lowering=False, debug=False)

    xt_d = nc.dram_tensor("xt16", [C, BC * T], FP16, kind="ExternalInput").ap()
    w_d = nc.dram_tensor("w16", [C, D], FP16, kind="ExternalInput").ap()
    ac_d = nc.dram_tensor("acoef", [4, 128, 400], F32, kind="ExternalInput").ap()
    bc_d = nc.dram_tensor("bcoef", [4, 128, 400], F32, kind="ExternalInput").ap()
    syn_d = nc.dram_tensor("syn", [D, BC * T], FP16, kind="ExternalOutput").ap()
    mem_d = nc.dram_tensor("mem", [D, BC * T], FP16, kind="ExternalOutput").ap()

    with tile.TileContext(nc) as tc:
        with ExitStack() as ctx:
            cpool = ctx.enter_context(tc.tile_pool(name="consts", bufs=1))
            # warm-up scratch zeroed on gpsimd (its queue is alive ~1us before
            # the DVE's first op can land), issued ahead of the w DMAs
            warm_sb = cpool.tile([128, 512], FP16, name="warm", tag="warm")
            nc.gpsimd.memset(warm_sb[:], 0.0)
            # weights + coefs on the gpsimd (SWDGE) queue so the Sync queue
            # leads with the first x tiles (A/B-tested vs the scalar queue:
            # scalar was ~5us slower end-to-end)
            w_tiles = []
            for k, (r0_, rk) in enumerate(KT):
                wt = cpool.tile([128, D], FP16, name=f"w{k}", tag=f"w{k}")
                nc.gpsimd.dma_start(wt[:rk, :], w_d[r0_ : r0_ + rk, :])
                w_tiles.append(wt)
            # coef tiles are small (one bank-width each; scans reuse them per
            # bank) so the startup DMA stays light
            ac_t, bc_t = [], []
            for di in range(4):
                a = cpool.tile([128, 400], F32, name=f"ac{di}", tag=f"ac{di}")
                nc.gpsimd.dma_start(a[:], ac_d[di])
                ac_t.append(a)
                b_ = cpool.tile([128, 400], F32, name=f"bc{di}", tag=f"bc{di}")
                nc.gpsimd.dma_start(b_[:], bc_d[di])
                bc_t.append(b_)

            xp = ctx.enter_context(tc.tile_pool(name="xp", bufs=2))
            pp = ctx.enter_context(tc.tile_pool(name="pp", bufs=2, space="PSUM"))
            sp = ctx.enter_context(tc.tile_pool(name="sp", bufs=2))
            vp = ctx.enter_context(tc.tile_pool(name="vp", bufs=2))
            hp = ctx.enter_context(tc.tile_pool(name="hp", bufs=2))

            # PE warmup: dummy matmuls run during the initial DMA wait so HAM
            # un-throttles before the first real MM
            warm_ps = pp.tile([128, 2048], F32, tag="ps", name="warm_ps")
            for _ in range(16):
                nc.tensor.matmul(
                    warm_ps[:, 0:512], warm_sb[:, 0:128], warm_sb[:], start=True, stop=True
                )

            for q in range(NQ):
                qc0 = q * QCOLS
                xts = []
                for k, (r0_, rk) in enumerate(KT):
                    t_ = xp.tile([128, QCOLS], FP16, tag=f"x{k}", name=f"x{k}_{q}")
                    nc.sync.dma_start(t_[:rk, :], xt_d[r0_ : r0_ + rk, qc0 : qc0 + QCOLS])
                    xts.append(t_)

                for di in range(4):
                    dsl = slice(di * 128, (di + 1) * 128)

                    # h matmul: 4 batches per PSUM bank, shifted write to t+1.
                    # The t=0 slots are zeroed by an extra k=0 matmul over the
                    # host-zeroed t=99 x columns (avoids a DVE memset + stall).
                    ps = pp.tile([128, 2048], F32, tag="ps", name=f"ps_{q}_{di}")
                    for k, (r0_, rk) in enumerate(KT):
                        lhsT = w_tiles[k][:rk, dsl]
                        for g in range(4):
                            if sim_safe:
                                for b_ in range(4):
                                    c0 = g * 400 + b_ * 100
                                    nc.tensor.matmul(
                                        ps[:, g * 512 + b_ * 100 + 1 : g * 512 + b_ * 100 + 100],
                                        lhsT,
                                        xts[k][:rk, c0 : c0 + 99],
                                        start=(k == 0 and b_ == 0),
                                        stop=(k == 5 and b_ == 3),
                                    )
                                    if k == 0:
                                        nc.tensor.matmul(
                                            ps[:, g * 512 + b_ * 100 : g * 512 + b_ * 100 + 1],
                                            lhsT,
                                            xts[k][:rk, c0 + 99 : c0 + 100],
                                            start=False, stop=False,
                                        )
                                continue
                            rhs3 = xts[k][:rk, g * 400 : (g + 1) * 400].rearrange(
                                "p (b t) -> p b t", t=100
                            )[:, :, 0:99]
                            out3 = ps[:, g * 512 : g * 512 + 400].rearrange(
                                "p (b t) -> p b t", t=100
                            )[:, :, 1:100]
                            nc.tensor.matmul(
                                out3, lhsT, rhs3, start=(k == 0), stop=(k == 5)
                            )
                            if k == 0:
                                z3 = ps[:, g * 512 : g * 512 + 400].rearrange(
                                    "p (b t) -> p b t", t=100
                                )[:, :, 0:1]
                                zr = xts[k][:rk, g * 400 : (g + 1) * 400].rearrange(
                                    "p (b t) -> p b t", t=100
                                )[:, :, 99:100]
                                nc.tensor.matmul(z3, lhsT, zr, start=False, stop=False)

                    # ACT stages h PSUM->SBUF (fp16): DVE scans with a PSUM
                    # input cost ~180ns more each than SBUF-input scans, and
                    # the ACT engine is otherwise idle (copy overlaps the
                    # previous unit's scans)
                    h16 = hp.tile([128, QCOLS], FP16, tag="h16", name=f"h16_{q}_{di}")
                    nc.scalar.activation(
                        h16.rearrange("p (g c) -> p g c", c=400),
                        ps.rearrange("p (g x) -> p g x", x=512)[:, :, 0:400],
                        COPY,
                    )
                    syn16 = sp.tile([128, QCOLS], FP16, tag="syn", name=f"sy_{q}_{di}")
                    v16 = vp.tile([128, QCOLS], FP16, tag="v", name=f"v_{q}_{di}")
                    for g in range(4):
                        nc.vector.tensor_tensor_scan(
                            syn16[:, g * 400 : (g + 1) * 400],
                            ac_t[di][:],
                            h16[:, g * 400 : (g + 1) * 400],
                            0.0,
                            MULT,
                            ADD,
                        )
                    nc.scalar.dma_start(syn_d[dsl, qc0 : qc0 + QCOLS], syn16[:])
                    last_unit = q == NQ - 1 and di == 3
                    for g in range(4):
                        nc.vector.tensor_tensor_scan(
                            v16[:, g * 400 : (g + 1) * 400],
                            bc_t[di][:],
                            syn16[:, g * 400 : (g + 1) * 400],
                            0.0,
                            MULT,
                            ADD,
                        )
                        if last_unit:
                            # quarter-stores so the post-final-scan drain is
                            # one 102 KB transfer instead of 409 KB
                            nc.scalar.dma_start(
                                mem_d[dsl, qc0 + g * 400 : qc0 + (g + 1) * 400],
                                v16[:, g * 400 : (g + 1) * 400],
                            )
                    if not last_unit:
                        nc.scalar.dma_start(mem_d[dsl, qc0 : qc0 + QCOLS], v16[:])

    nc.compile()
    _cache[key] = nc
    return nc


def kernel(inputs, w, alpha, beta):
    global LAST_RESULT
    inputs = np.asarray(inputs, dtype=np.float32)
    w = np.asarray(w, dtype=np.float32)
    alpha = np.asarray(alpha, dtype=np.float32).reshape(-1)
    beta = np.asarray(beta, dtype=np.float32).reshape(-1)

    nc = _build()

    acoef = np.broadcast_to(
        alpha.reshape(4, 128, 1), (4, 128, 400)
    ).astype(np.float32).copy()
    acoef[:, :, 0::100] = 0.0
    bcoef = np.broadcast_to(
        beta.reshape(4, 128, 1), (4, 128, 400)
    ).astype(np.float32).copy()
    bcoef[:, :, 0::100] = 0.0
    w16 = w.astype(np.float16)
    omb = (1.0 - beta).reshape(1, 1, D)

    in_maps = []
    for c in range(NCORES):
        xc = inputs[c * BC : (c + 1) * BC].copy()  # [64, 100, 700]
        xc[:, T - 1, :] = 0.0  # t=99 cols feed the PSUM t=0 zeroing matmul
        xt16 = xc.reshape(BC * T, C).T.astype(np.float16)  # [700, 6400]
        in_maps.append({"xt16": xt16, "w16": w16, "acoef": acoef, "bcoef": bcoef})

    run_kwargs = {}
    if os.environ.get("MEMBRANE_TRACE_DIR"):
        run_kwargs["tmpdir"] = os.environ["MEMBRANE_TRACE_DIR"]
    res = bass_utils.run_bass_kernel_spmd(
        nc, in_maps, core_ids=list(range(NCORES)), **run_kwargs
    )
    LAST_RESULT = res

    syn_full = np.empty((B, T, D), dtype=np.float32)
    mem_full = np.empty((B, T, D), dtype=np.float32)
    for c in range(NCORES):
        r = res.results[c]
        cs = slice(c * BC, (c + 1) * BC)
        syn_full[cs] = (
            r["syn"].astype(np.float32).reshape(D, BC, T).transpose(1, 2, 0)
        )
        vt = r["mem"].astype(np.float32).reshape(D, BC, T).transpose(1, 2, 0)
        mem_full[cs, 1:, :] = vt[:, : T - 1, :] * omb
    syn_full[:, 0, :] = 0.0
    mem_full[:, 0, :] = 0.0
    return (syn_full, mem_full)
